# revision 24
# baseline (speedup 1.0000x reference)
"""Trainium2 Bass kernel for a dense transformer block.

Strategy: data-parallel over batch (8 batch elems -> 8 cores, no collectives).
Per core: x[1024, 1024] through LN1 -> qkv -> attention -> proj(+res) -> LN2 ->
fc1 -> gelu -> fc2(+res). Matmuls in fp8 DoubleRow where possible with fp32
PSUM accumulation.  LayerNorm gamma/beta are folded into the following matmul's
weights/bias on the host, and the attention scale 1/sqrt(hd) is folded into the
q-part of the qkv weights.

Attention uses a transposed-scores layout: S^T[m, n] tiles come straight out of
the PE with keys (m) on partitions, exp() is applied on eviction (no max
subtraction needed: inputs are layernormed, |scores| is O(1)), and the P^T @ v
matmul contracts m on partitions.  The two heads of a pair live in partition
rows 0-63 / 64-127 so their score / AV matmuls dual-issue in the PE array.
The attention inner loop is software-pipelined at m-chunk granularity with the
exp() evictions on the scalar engine as the pacing resource; softmax column
sums are split between DVE (head 0) and GpSimd (head 1) so no single engine
outruns the exp stream.  PSUM is partitioned into a dedicated AV accumulator
(2 banks) and a 6-bank rotation shared by score tiles, filler qkv/v chains and
the softmax-denominator broadcast.  proj -> LN2 -> fc1 are fused into one
pipelined loop (LN2 transposes lag one tile so the PE never waits on the
DVE normalize chain), and LN normalizes alternate between the scalar and
vector engines.
"""

import numpy as np
import ml_dtypes

B = 8
T = 1024
D = 1024
H = 16
HD = D // H
FF = 4096
EPS = 1e-5
P = 128
N_CORES = 8

NT = T // P      # 8 token tiles
KD = D // P      # 8 contraction chunks over d
NF = 3 * D // P  # 24 qkv feature tiles
NFF = FF // P    # 32 ff feature tiles
HPF = P // HD    # 2 heads per 128-feature tile

_CACHE = {}


def _build_nc():
    from contextlib import ExitStack

    import concourse.bass as bass
    import concourse.mybir as mybir
    import concourse.tile as tile
    from concourse import bacc
    from concourse.masks import make_identity

    dt = mybir.dt
    f32, bf16, f8 = dt.float32, dt.bfloat16, dt.float8e4
    AF = mybir.ActivationFunctionType
    ALU = mybir.AluOpType
    DR = mybir.MatmulPerfMode.DoubleRow

    nc = bacc.Bacc("TRN2", target_bir_lowering=False, debug=False,
                   num_devices=N_CORES)

    # weights come pre-packed partition-major from the host so every SBUF
    # weight tile loads with one DMA of large contiguous per-partition lines
    xb = nc.dram_tensor("xb", [T, D], f32, kind="ExternalInput").ap()
    wqkv_pk = nc.dram_tensor("wqkv_pk", [P, 2 * KD, KD, P], f8,
                             kind="ExternalInput").ap()
    wv_pk = nc.dram_tensor("wv_pk", [P, KD, D], f8,
                           kind="ExternalInput").ap()
    bqkv = nc.dram_tensor("bqkv", [P, NF], f32, kind="ExternalInput").ap()
    wp_pk = nc.dram_tensor("wp_pk", [P, KD, D], f8,
                           kind="ExternalInput").ap()
    wfc1_pk = nc.dram_tensor("wfc1_pk", [P, NFF, KD, P], f8,
                             kind="ExternalInput").ap()
    bfc1 = nc.dram_tensor("bfc1", [P, NFF], f32, kind="ExternalInput").ap()
    wfc2_pk = nc.dram_tensor("wfc2_pk", [P, 2, NFF, 512], f8,
                             kind="ExternalInput").ap()
    bfc2 = nc.dram_tensor("bfc2", [P, D], f32, kind="ExternalInput").ap()
    xb2 = nc.dram_tensor("xb2", [T, D], f32, kind="ExternalInput").ap()
    out = nc.dram_tensor("out", [T, D], f32, kind="ExternalOutput").ap()

    with tile.TileContext(nc) as tc, ExitStack() as ctx:
        const = ctx.enter_context(tc.tile_pool(name="const", bufs=1))
        eps_t = const.tile([P, 1], f32)
        nc.vector.memset(eps_t, EPS)

        ident = const.tile([P, P], bf16)
        make_identity(nc, ident)
        ones_mat = const.tile([P, 64], bf16)
        nc.vector.memset(ones_mat, 1.0)

        bqkv_sb = const.tile([P, NF], f32)
        bfc1_sb = const.tile([P, NFF], f32)
        bfc2_bc = const.tile([P, D], f32)

        def ln_stats_norm(src_pool, stat_pool, x_t, norm_on_act=False):
            """x_t [P, D] f32 -> normalized bf16 tile (returned).

            Stats on DVE; optionally the normalize runs on ACT (Copy with a
            per-token scale after a DVE-side subtract) to split the LN load.
            """
            st = stat_pool.tile([P, 2, 6], f32, name="st")
            xr = x_t.rearrange("p (s q) -> p s q", s=2)
            nc.vector.bn_stats(out=st[:, 0, :], in_=xr[:, 0, :])
            nc.vector.bn_stats(out=st[:, 1, :], in_=xr[:, 1, :])
            mv = stat_pool.tile([P, 2], f32, name="mv")
            nc.vector.bn_aggr(out=mv, in_=st)
            rstd = stat_pool.tile([P, 1], f32, name="rstd")
            nc.scalar.activation(out=rstd, in_=mv[:, 1:2], func=AF.Sqrt,
                                 bias=eps_t)
            nc.vector.reciprocal(out=rstd, in_=rstd)
            h_bf = src_pool.tile([P, D], bf16, name="h_bf")
            if norm_on_act:
                nmr = stat_pool.tile([P, 1], f32, name="nmr")
                nc.vector.scalar_tensor_tensor(
                    out=nmr, in0=mv[:, 0:1], scalar=-1.0, in1=rstd,
                    op0=ALU.mult, op1=ALU.mult)
                nc.scalar.activation(out=h_bf, in_=x_t, func=AF.Identity,
                                     scale=rstd, bias=nmr)
            else:
                nc.vector.tensor_scalar(out=h_bf, in0=x_t,
                                        scalar1=mv[:, 0:1], scalar2=rstd,
                                        op0=ALU.subtract, op1=ALU.mult)
            return h_bf

        def ln_transpose(ps_pool, h_bf, dst_tile, it):
            # transpose via regular matmul against identity (out = h^T @ I):
            # all 8 chunks land in one 2-bank psum tile and leave through a
            # single 1024-wide scalar-engine copy.  Callers lag this one
            # tile behind the normalize so the PE never waits on it.
            pt = ps_pool.tile([P, T], f32, name="ps_mm")
            for kd in range(KD):
                nc.tensor.matmul(pt[:, kd * P:(kd + 1) * P],
                                 h_bf[:, kd * P:(kd + 1) * P], ident,
                                 start=True, stop=True)
            nc.scalar.copy(
                out=dst_tile[:, :, it * P:(it + 1) * P],
                in_=pt.rearrange("p (q t) -> p q t", q=KD))

        # Pool nesting is by lifetime (LIFO): x2 dies last, then w2a, oT, ...
        with tc.tile_pool(name="phX2", bufs=1) as phX2:
          x2 = phX2.tile([P, NT, D], f32)
          with tc.tile_pool(name="phOT", bufs=1) as phOT, \
               tc.tile_pool(name="phB", bufs=1) as phB, \
               tc.tile_pool(name="wpP", bufs=1) as wpP, \
               tc.tile_pool(name="xr", bufs=3) as xr_pool, \
               tc.tile_pool(name="stat2", bufs=6) as stat2_pool:
            oT = phOT.tile([P, KD, T], f8)
            qkvT = phB.tile([P, 2 * KD, T], bf16)
            wp_sb = wpP.tile([P, KD, D], f8)
            with tc.tile_pool(name="phC", bufs=1) as phC:
              v_tok = phC.tile([P, NT, D], f8)
              wv_sb = phC.tile([P, KD, D], f8)

              # ---------------- Phase A: LN1 -> hT, fused with the
              # attention prologue (pair-0 q/k chains and the first v pair
              # interleave into the LN loop once their token half is ready)
              # and phases B-D: software-pipelined attention over head
              # pairs. -------------
              with tc.tile_pool(name="phA", bufs=1) as phA:
                hT2 = [phA.tile([P, KD, T // 2], f8, name="hTa"),
                       phA.tile([P, KD, T // 2], f8, name="hTb")]
                with tc.tile_pool(name="xa", bufs=5) as xa_pool, \
                     tc.tile_pool(name="stat", bufs=6) as stat_pool, \
                     tc.tile_pool(name="wq", bufs=6) as wq_pool, \
                     tc.tile_pool(name="ptp", bufs=2) as pt_pool, \
                     tc.tile_pool(name="attn_sm", bufs=1) as sm_pool, \
                     tc.tile_pool(name="ps_sc", bufs=3,
                                  space="PSUM") as ps_sc, \
                     tc.tile_pool(name="ps_avp", bufs=1,
                                  space="PSUM") as ps_avp:

                    def emit_qkv_ft(ft):
                        # q/k feature tile: fp8 DoubleRow (256-contraction)
                        # chains; both token halves accumulate in one 2-bank
                        # psum tile, single 1024-wide descale+bias evict
                        w_t = wq_pool.tile([P, KD, P], f8, name="w_t")
                        nc.sync.dma_start(out=w_t, in_=wqkv_pk[:, ft])
                        ps = ps_sc.tile([P, T], f32, name="ps_mm")
                        for nh in range(2):
                            for k in range(0, KD, 2):
                                nc.tensor.matmul(
                                    ps[:, nh * 512:(nh + 1) * 512],
                                    w_t[:, k:k + 2, :],
                                    hT2[nh][:, k:k + 2, :],
                                    start=(k == 0), stop=(k == KD - 2),
                                    perf_mode=DR)
                        descale = 2.0 ** -10 if ft < KD else 2.0 ** -7
                        nc.vector.tensor_scalar(
                            out=qkvT[:, ft, :],
                            in0=ps, scalar1=descale,
                            scalar2=bqkv_sb[:, ft:ft + 1], op0=ALU.mult,
                            op1=ALU.add)

                    def emit_qkv_half(ft, nh):
                        # single token-half q/k chain for the phase-A
                        # prologue (only needs hT2[nh])
                        w_t = wq_pool.tile([P, KD, P], f8, name="w_t")
                        nc.sync.dma_start(out=w_t, in_=wqkv_pk[:, ft])
                        ps = ps_sc.tile([P, T], f32, name="ps_mm")
                        for k in range(0, KD, 2):
                            nc.tensor.matmul(
                                ps[:, 0:512],
                                w_t[:, k:k + 2, :],
                                hT2[nh][:, k:k + 2, :],
                                start=(k == 0), stop=(k == KD - 2),
                                perf_mode=DR)
                        descale = 2.0 ** -10 if ft < KD else 2.0 ** -7
                        nc.vector.tensor_scalar(
                            out=qkvT[:, ft, nh * 512:(nh + 1) * 512],
                            in0=ps[:, 0:512], scalar1=descale,
                            scalar2=bqkv_sb[:, ft:ft + 1], op0=ALU.mult,
                            op1=ALU.add)

                    def emit_v_pair(g, tt):
                        # v token-major for token tiles (tt, tt+1) of the
                        # 512-wide feature chunk g (v_bias is folded into
                        # the proj bias)
                        vsl = slice(g * 512, g * 512 + 512)
                        ps = ps_sc.tile([P, T], f32, name="ps_mm")
                        for ti in range(2):
                            hTh = hT2[(tt + ti) // 4]
                            to = ((tt + ti) % 4) * P
                            for k in range(0, KD, 2):
                                nc.tensor.matmul(
                                    ps[:, ti * 512:(ti + 1) * 512],
                                    hTh[:, k:k + 2, to:to + P],
                                    wv_sb[:, k:k + 2, vsl],
                                    start=(k == 0), stop=(k == KD - 2),
                                    perf_mode=DR)
                        nc.vector.tensor_scalar(
                            out=v_tok[:, tt:tt + 2, vsl], in0=ps,
                            scalar1=2.0 ** -7, scalar2=None, op0=ALU.mult)

                    # bulk weight prefetches ride the gpsimd queue so the
                    # x tiles stream unimpeded on the sync queue
                    nc.gpsimd.dma_start(out=wv_sb, in_=wv_pk)
                    nc.gpsimd.dma_start(out=bqkv_sb, in_=bqkv)

                    # LN1 over token tiles; transposes lag one tile so the
                    # PE issue stream never parks behind the DVE chain
                    def x_dma(x_t, it):
                        # halves land in parallel on two DMA-capable queues
                        # (halves the per-tile latency; the first bn_stats
                        # chunk only needs columns 0-511)
                        nc.sync.dma_start(
                            out=x_t[:, 0:512],
                            in_=xb[it * P:(it + 1) * P, 0:512])
                        nc.scalar.dma_start(
                            out=x_t[:, 512:D],
                            in_=xb[it * P:(it + 1) * P, 512:D])

                    h_prev = None
                    x_ts = {}
                    for it in range(2):
                        x_ts[it] = xa_pool.tile([P, D], f32, name="x_t")
                        x_dma(x_ts[it], it)
                    for it in range(NT):
                        if it in x_ts:
                            x_t = x_ts.pop(it)
                        else:
                            x_t = xa_pool.tile([P, D], f32, name="x_t")
                            x_dma(x_t, it)
                        h_cur = ln_stats_norm(xa_pool, stat_pool, x_t,
                                              norm_on_act=(it % 2 == 1))
                        if h_prev is not None:
                            ln_transpose(ps_sc, h_prev, hT2[(it - 1) // 4],
                                         (it - 1) % 4)
                        h_prev = h_cur
                        if it == 4:
                            emit_qkv_half(0, 0)
                            emit_qkv_half(KD, 0)
                        if it == 5:
                            emit_v_pair(0, 0)
                        if it == 6:
                            emit_v_pair(0, 2)
                        if it == 7:
                            emit_v_pair(0, 4)
                    ln_transpose(ps_sc, h_prev, hT2[1], 3)

                    state = {}

                    def av_tail(p):
                        # last AV chunk of pair p, deferred into pair p+1's
                        # first slot so it doesn't hold up that pair's scores
                        u4s, ps_av, PTl = state[p]
                        PTl = PTl()
                        for ch in range(2):
                            for j in range(HPF):
                                po = j * HD
                                hf = (HPF * p + j) * HD
                                nc.tensor.matmul(
                                    ps_av[po:po + HD,
                                          ch * 512:(ch + 1) * 512],
                                    v_tok[:, NT - 1, hf:hf + HD],
                                    PTl[j][:, ch * 512:(ch + 1) * 512],
                                    start=False, stop=True)

                    def finalize_pair(p):
                        # ones-matmuls broadcast this pair's column sums into
                        # 64 partitions per head of a 2-bank bc tile; recip +
                        # 1024-wide normalize eviction follow.
                        u4s, ps_av, _ = state[p]
                        bc = ps_sc.tile([P, T], f32, name="ps_mm")
                        for j in range(HPF):
                            po = j * HD
                            for ch in range(2):
                                nc.tensor.matmul(
                                    bc[po:po + HD, ch * 512:(ch + 1) * 512],
                                    ones_mat[:, 0:HD],
                                    u4s[j][:, ch * 512:(ch + 1) * 512],
                                    start=True, stop=True)
                        rec_sb = sm_pool.tile([P, T], f32, name="rec_sb")
                        nc.vector.reciprocal_approx_fast(out=rec_sb, in_=bc)
                        nc.vector.tensor_tensor(
                            out=oT[:, p, :], in0=ps_av, in1=rec_sb,
                            op=ALU.mult)

                    # prologue tail: pair 0's second token halves
                    emit_qkv_half(0, 1)
                    emit_qkv_half(KD, 1)

                    # filler chains per pair: qkv tiles for pair p+1, v
                    # chunks just-in-time for this group's AV consumption
                    fillers = {p: [] for p in range(H // HPF)}
                    fillers[0] = [lambda: emit_v_pair(0, 6)]
                    for p in range(H // HPF - 1):
                        fillers[p] += [
                            lambda ft=p + 1: emit_qkv_half(ft, 0),
                            lambda ft=KD + p + 1: emit_qkv_half(ft, 0),
                            lambda ft=p + 1: emit_qkv_half(ft, 1),
                            lambda ft=KD + p + 1: emit_qkv_half(ft, 1)]
                    fillers[2] += [lambda: emit_v_pair(1, 0)]
                    fillers[3] += [lambda: emit_v_pair(1, 2),
                                   lambda: emit_v_pair(1, 4)]
                    fillers[4] += [lambda: emit_v_pair(1, 6)]

                    for p in range(H // HPF):
                        ft_q = p
                        ft_k = KD + p
                        PT = {}
                        u4s = [sm_pool.tile([P, T], bf16, name=f"u4_{j}")
                               for j in range(HPF)]
                        ps_av = None
                        for mt in range(NT):
                            for j in range(HPF):
                                po = j * HD
                                ps_s = ps_sc.tile([P, T], f32, name="ps_mm")
                                for ch in range(2):
                                    nc.tensor.matmul(
                                        ps_s[:, ch * 512:(ch + 1) * 512],
                                        qkvT[po:po + HD, ft_k,
                                             mt * P:(mt + 1) * P],
                                        qkvT[po:po + HD, ft_q,
                                             ch * 512:(ch + 1) * 512],
                                        start=True, stop=True)
                                # per-chunk PT ring (4 deep per head) so the
                                # next pair's exp never waits on this pair's
                                # AV tail; 1024-wide exp amortizes ACT cost
                                PT[(j, mt)] = pt_pool.tile(
                                    [P, T], bf16, name=f"PT{j}", bufs=4)
                                nc.scalar.activation(
                                    out=PT[(j, mt)], in_=ps_s,
                                    func=AF.Exp)
                                if mt == 1:
                                    nc.vector.tensor_tensor(
                                        out=u4s[j], in0=PT[(j, 0)],
                                        in1=PT[(j, 1)], op=ALU.add)
                                elif mt > 1:
                                    nc.vector.tensor_tensor(
                                        out=u4s[j], in0=u4s[j],
                                        in1=PT[(j, mt)], op=ALU.add)
                            if mt == 0 and p > 0:
                                av_tail(p - 1)
                                finalize_pair(p - 1)
                            if mt == 5 and p == 4:
                                # prefetch the proj weights on the
                                # otherwise-idle sync queue mid-attention
                                nc.sync.dma_start(out=wp_sb, in_=wp_pk)
                            if 1 <= mt <= len(fillers[p]):
                                fillers[p][mt - 1]()
                            elif mt == 7 and len(fillers[p]) == 7:
                                fillers[p][6]()
                            if mt > 0:
                                if ps_av is None:
                                    ps_av = ps_avp.tile([P, T], f32,
                                                        name="ps_av")
                                    state[p] = (
                                        u4s, ps_av,
                                        lambda PT=PT: [PT[(j, NT - 1)]
                                                       for j in range(HPF)])
                                for ch in range(2):
                                    for j in range(HPF):
                                        po = j * HD
                                        hf = (HPF * p + j) * HD
                                        nc.tensor.matmul(
                                            ps_av[po:po + HD,
                                                  ch * 512:(ch + 1) * 512],
                                            v_tok[:, mt - 1, hf:hf + HD],
                                            PT[(j, mt - 1)][
                                                :, ch * 512:(ch + 1) * 512],
                                            start=(mt == 1), stop=False)
                    av_tail(H // HPF - 1)
                    finalize_pair(H // HPF - 1)
                    # preload the sqrt ACT table set while the scalar
                    # engine is idle so LN2's first rstd isn't behind a
                    # 1.3us table switch
                    warm = stat2_pool.tile([P, 1], f32, name="warm")
                    nc.scalar.activation(out=warm, in_=eps_t, func=AF.Sqrt,
                                         bias=eps_t)

            # -------- Phases E2+F fused: proj + residual -> x2, LN2 and
            # the first fc1 half-chains pipelined per token tile (qkvT/v/o
            # freed when the enclosing pools close).  LN2 transposes lag one
            # tile behind the proj/normalize chain so the PE stream never
            # parks on a cross-engine dependency. ------
            with tc.tile_pool(name="phG2", bufs=1) as phG2:
              aT = phG2.tile([P, NFF, T], f8)
              w2a = phG2.tile([P, NFF, 512], f8, name="w2a")
              nc.gpsimd.dma_start(out=bfc1_sb, in_=bfc1)
              nc.gpsimd.dma_start(out=bfc2_bc, in_=bfc2)
              nc.gpsimd.dma_start(out=w2a, in_=wfc2_pk[:, 0])
              with tc.tile_pool(name="phF", bufs=1) as phF:
                h2T2 = [phF.tile([P, KD, T // 2], f8, name="h2Ta"),
                        phF.tile([P, KD, T // 2], f8, name="h2Tb")]
                NEARLY = 8

                with tc.tile_pool(name="w1h", bufs=4) as w1h_pool, \
                     tc.tile_pool(name="ps_e", bufs=3,
                                  space="PSUM") as ps_e, \
                     tc.tile_pool(name="ps_f1h", bufs=2,
                                  space="PSUM") as ps_f1h:

                    def fc1_half(w_pool, ps_pool, ff, nh):
                        w_t = w_pool.tile([P, KD, P], f8, name="w1_t")
                        nc.sync.dma_start(out=w_t, in_=wfc1_pk[:, ff])
                        ps = ps_pool.tile([P, 512], f32, name="ps_half",
                                          bufs=2)
                        for k in range(0, KD, 2):
                            nc.tensor.matmul(
                                ps, w_t[:, k:k + 2, :],
                                h2T2[nh][:, k:k + 2, :],
                                start=(k == 0), stop=(k == KD - 2),
                                perf_mode=DR)
                        nc.scalar.activation(
                            out=aT[:, ff, nh * 512:(nh + 1) * 512],
                            in_=ps, func=AF.Gelu, scale=2.0 ** -7,
                            bias=bfc1_sb[:, ff:ff + 1])

                    # proj sweep first: with only the cheap evictions on
                    # DVE the proj stream runs PE-bound back-to-back; the
                    # LN2 stats chains (the expensive DVE work) follow in a
                    # second pipelined sweep so they never gate the proj
                    # PSUM ring
                    x_rs = {}
                    for tt in range(3):
                        x_rs[tt] = xr_pool.tile([P, D], f32, name="x_r")
                        nc.sync.dma_start(out=x_rs[tt],
                                          in_=xb2[tt * P:(tt + 1) * P, :])
                    for tt in range(NT):
                        if tt in x_rs:
                            x_r = x_rs.pop(tt)
                        else:
                            x_r = xr_pool.tile([P, D], f32, name="x_r")
                            nc.sync.dma_start(out=x_r,
                                              in_=xb2[tt * P:(tt + 1) * P, :])
                        ps = ps_e.tile([P, T], f32, name="ps_mm")
                        for dh in range(2):
                            for k in range(0, KD, 2):
                                nc.tensor.matmul(
                                    ps[:, dh * 512:(dh + 1) * 512],
                                    oT[:, k:k + 2, tt * P:(tt + 1) * P],
                                    wp_sb[:, k:k + 2,
                                          dh * 512:(dh + 1) * 512],
                                    start=(k == 0), stop=(k == KD - 2),
                                    perf_mode=DR)
                        # x_r already carries x + proj bias (host-folded),
                        # so proj evict + bias + residual is one DVE op
                        nc.vector.scalar_tensor_tensor(
                            out=x2[:, tt, :], in0=ps, scalar=2.0 ** -7,
                            in1=x_r, op0=ALU.mult, op1=ALU.add)
                    h2_prev = None
                    for tt in range(NT):
                        h2_cur = ln_stats_norm(xr_pool, stat2_pool,
                                               x2[:, tt, :])
                        if h2_prev is not None:
                            ln_transpose(ps_e, h2_prev, h2T2[(tt - 1) // 4],
                                         (tt - 1) % 4)
                        h2_prev = h2_cur
                        if tt >= 4:
                            fc1_half(w1h_pool, ps_f1h, 2 * (tt - 4), 0)
                            fc1_half(w1h_pool, ps_f1h, 2 * (tt - 4) + 1, 0)
                    ln_transpose(ps_e, h2_prev, h2T2[1], 3)

                # ------- Phase G: the rest of fc1 -------
                with tc.tile_pool(name="w1", bufs=6) as w1_pool, \
                     tc.tile_pool(name="ps_f1", bufs=3,
                                  space="PSUM") as ps_f1:
                    for ff in range(NFF):
                        if ff < NEARLY:
                            fc1_half(w1_pool, ps_f1, ff, 1)
                            continue
                        w_t = w1_pool.tile([P, KD, P], f8, name="w1_t")
                        nc.sync.dma_start(out=w_t, in_=wfc1_pk[:, ff])
                        ps = ps_f1.tile([P, T], f32, name="ps_mm")
                        for nh in range(2):
                            for k in range(0, KD, 2):
                                nc.tensor.matmul(
                                    ps[:, nh * 512:(nh + 1) * 512],
                                    w_t[:, k:k + 2, :],
                                    h2T2[nh][:, k:k + 2, :],
                                    start=(k == 0), stop=(k == KD - 2),
                                    perf_mode=DR)
                        nc.scalar.activation(
                            out=aT[:, ff, :],
                            in_=ps, func=AF.Gelu, scale=2.0 ** -7,
                            bias=bfc1_sb[:, ff:ff + 1])

              # ------- Phase H: fc2 + residual -> out (h2T2 freed) -------
              with tc.tile_pool(name="w2b", bufs=1) as w2b_pool, \
                   tc.tile_pool(name="yb", bufs=3) as y_pool, \
                   tc.tile_pool(name="ps_f2", bufs=4,
                                space="PSUM") as ps_f2:
                w2b = w2b_pool.tile([P, NFF, 512], f8, name="w2b")
                nc.gpsimd.dma_start(out=w2b, in_=wfc2_pk[:, 1])
                for dh, w2_t in ((0, w2a), (1, w2b)):
                    sl = slice(dh * 512, (dh + 1) * 512)
                    for tt in range(NT):
                        ps = ps_f2.tile([P, 512], f32, name="ps_mm")
                        for k in range(0, NFF, 2):
                            nc.tensor.matmul(
                                ps, aT[:, k:k + 2, tt * P:(tt + 1) * P],
                                w2_t[:, k:k + 2, :],
                                start=(k == 0), stop=(k == NFF - 2),
                                perf_mode=DR)
                        y_sb = y_pool.tile([P, 512], f32, name="y_sb")
                        nc.vector.scalar_tensor_tensor(
                            out=y_sb, in0=ps, scalar=2.0 ** -8,
                            in1=bfc2_bc[:, sl], op0=ALU.mult, op1=ALU.add)
                        nc.vector.tensor_tensor(
                            out=y_sb, in0=y_sb,
                            in1=x2[:, tt, sl], op=ALU.add)
                        dma_eng = nc.sync if (tt + dh) % 2 == 0 \
                            else nc.gpsimd
                        dma_eng.dma_start(
                            out=out[tt * P:(tt + 1) * P, sl],
                            in_=y_sb)

    nc.compile()
    return nc


def _prep_host_inputs(x, ln1_g, ln1_b, ln2_g, ln2_b, qkv_w, q_bias, v_bias,
                      proj_w, proj_b, fc1_w, fc1_b, fc2_w, fc2_b):
    f32 = np.float32
    x = np.asarray(x, f32)
    ln1_g = np.asarray(ln1_g, f32)
    ln1_b = np.asarray(ln1_b, f32)
    ln2_g = np.asarray(ln2_g, f32)
    ln2_b = np.asarray(ln2_b, f32)
    qkv_w = np.asarray(qkv_w, f32)
    q_bias = np.asarray(q_bias, f32)
    v_bias = np.asarray(v_bias, f32)
    proj_w = np.asarray(proj_w, f32)
    proj_b = np.asarray(proj_b, f32)
    fc1_w = np.asarray(fc1_w, f32)
    fc1_b = np.asarray(fc1_b, f32)
    fc2_w = np.asarray(fc2_w, f32)
    fc2_b = np.asarray(fc2_b, f32)

    scale = HD ** (-0.5)
    # v_bias is a constant per-feature shift of o (softmax rows sum to 1),
    # so it folds into the proj bias: proj_b += proj_w @ v_bias.  The v part
    # of the qkv bias used on-chip is qkv_w_v @ ln1_b only.
    qkv_bias = np.concatenate(
        [q_bias, np.zeros_like(v_bias), np.zeros_like(v_bias)])
    wqkv = qkv_w * ln1_g[None, :]
    bqkv = qkv_w @ ln1_b + qkv_bias
    wqkv = wqkv.copy()
    wqkv[:D] *= scale
    bqkv[:D] *= scale
    proj_b = proj_b + proj_w @ (qkv_w[2 * D:] @ ln1_b + v_bias)

    wfc1 = fc1_w * ln2_g[None, :]
    bfc1 = fc1_w @ ln2_b + fc1_b

    # qkv/fc1/fc2 weights are cast to fp8e4m3 with power-of-two pre-scales
    # (q rows carry an extra 1/8 from the attention scale fold); the kernel
    # multiplies the matching 2^-s back in during psum eviction.  All weight
    # matrices are repacked partition-major so each SBUF tile loads with one
    # DMA of large contiguous per-partition lines.
    f8 = ml_dtypes.float8_e4m3
    KD_, NFF_, NF_ = D // 128, FF // 128, 3 * D // 128
    qkv_colscale = np.concatenate(
        [np.full(D, 2.0 ** 10, f32), np.full(2 * D, 2.0 ** 7, f32)])
    wqkvT8 = (wqkv.T * qkv_colscale[None, :]).astype(f8)
    wqkv_pk = wqkvT8[:, :2 * D].reshape(KD_, 128, 2 * KD_, 128) \
        .transpose(1, 2, 0, 3)
    wv_pk = wqkvT8[:, 2 * D:].reshape(KD_, 128, D).transpose(1, 0, 2)
    wp_pk = (proj_w.T * 2.0 ** 7).astype(f8) \
        .reshape(KD_, 128, D).transpose(1, 0, 2)
    wfc1_pk = (wfc1.T * 2.0 ** 7).astype(f8) \
        .reshape(KD_, 128, NFF_, 128).transpose(1, 2, 0, 3)
    wfc2_pk = (fc2_w.T * 2.0 ** 8).astype(f8) \
        .reshape(NFF_, 128, 2, 512).transpose(1, 2, 0, 3)
    shared = {
        "wqkv_pk": np.ascontiguousarray(wqkv_pk),
        "wv_pk": np.ascontiguousarray(wv_pk),
        "bqkv": np.ascontiguousarray(bqkv.reshape(NF_, 128).T, f32),
        "wp_pk": np.ascontiguousarray(wp_pk),
        "wfc1_pk": np.ascontiguousarray(wfc1_pk),
        "bfc1": np.ascontiguousarray(bfc1.reshape(NFF_, 128).T, f32),
        "wfc2_pk": np.ascontiguousarray(wfc2_pk),
        "bfc2": np.ascontiguousarray(
            np.broadcast_to(fc2_b, (128, D)), f32),
    }
    # residual stream with the proj bias pre-added (saves an on-chip add)
    x2r = x + proj_b[None, None, :]
    in_maps = [dict(shared, xb=np.ascontiguousarray(x[i]),
                    xb2=np.ascontiguousarray(x2r[i], f32))
               for i in range(N_CORES)]
    return in_maps


def kernel(**inputs):
    from concourse.bass_utils import run_bass_kernel_spmd

    if "nc" not in _CACHE:
        _CACHE["nc"] = _build_nc()
    nc = _CACHE["nc"]
    in_maps = _prep_host_inputs(**inputs)
    res = run_bass_kernel_spmd(nc, in_maps, core_ids=list(range(N_CORES)),
                               trace=False)
    return np.stack([res.results[i]["out"] for i in range(N_CORES)], axis=0)


if __name__ == "__main__":
    rng = np.random.default_rng(0)
    ins = {
        "x": rng.standard_normal((B, T, D)).astype(np.float32),
        "ln1_g": np.ones(D, np.float32), "ln1_b": np.zeros(D, np.float32),
        "ln2_g": np.ones(D, np.float32), "ln2_b": np.zeros(D, np.float32),
        "qkv_w": (rng.uniform(-1, 1, (3 * D, D)) / 32).astype(np.float32),
        "q_bias": np.zeros(D, np.float32), "v_bias": np.zeros(D, np.float32),
        "proj_w": (rng.uniform(-1, 1, (D, D)) / 32).astype(np.float32),
        "proj_b": np.zeros(D, np.float32),
        "fc1_w": (rng.uniform(-1, 1, (FF, D)) / 32).astype(np.float32),
        "fc1_b": np.zeros(FF, np.float32),
        "fc2_w": (rng.uniform(-1, 1, (D, FF)) / 64).astype(np.float32),
        "fc2_b": np.zeros(D, np.float32),
    }
    y = kernel(**ins)
    print("out", y.shape, y.dtype, np.abs(y).max())


# revision 25
# speedup vs baseline: 1.0138x; 1.0138x over previous
"""Trainium2 Bass kernel for a dense transformer block.

Strategy: data-parallel over batch (8 batch elems -> 8 cores, no collectives).
Per core: x[1024, 1024] through LN1 -> qkv -> attention -> proj(+res) -> LN2 ->
fc1 -> gelu -> fc2(+res). Matmuls in fp8 DoubleRow where possible with fp32
PSUM accumulation.  LayerNorm gamma/beta are folded into the following matmul's
weights/bias on the host, and the attention scale 1/sqrt(hd) is folded into the
q-part of the qkv weights.

Attention uses a transposed-scores layout: S^T[m, n] tiles come straight out of
the PE with keys (m) on partitions, exp() is applied on eviction (no max
subtraction needed: inputs are layernormed, |scores| is O(1)), and the P^T @ v
matmul contracts m on partitions.  The two heads of a pair live in partition
rows 0-63 / 64-127 so their score / AV matmuls dual-issue in the PE array.
The attention inner loop is software-pipelined at m-chunk granularity with the
exp() evictions on the scalar engine as the pacing resource; softmax column
sums are split between DVE (head 0) and GpSimd (head 1) so no single engine
outruns the exp stream.  PSUM is partitioned into a dedicated AV accumulator
(2 banks) and a 6-bank rotation shared by score tiles, filler qkv/v chains and
the softmax-denominator broadcast.  proj -> LN2 -> fc1 are fused into one
pipelined loop (LN2 transposes lag one tile so the PE never waits on the
DVE normalize chain), and LN normalizes alternate between the scalar and
vector engines.
"""

import numpy as np
import ml_dtypes

B = 8
T = 1024
D = 1024
H = 16
HD = D // H
FF = 4096
EPS = 1e-5
P = 128
N_CORES = 8

NT = T // P      # 8 token tiles
KD = D // P      # 8 contraction chunks over d
NF = 3 * D // P  # 24 qkv feature tiles
NFF = FF // P    # 32 ff feature tiles
HPF = P // HD    # 2 heads per 128-feature tile

_CACHE = {}


def _build_nc():
    from contextlib import ExitStack

    import concourse.bass as bass
    import concourse.mybir as mybir
    import concourse.tile as tile
    from concourse import bacc
    from concourse.masks import make_identity

    dt = mybir.dt
    f32, bf16, f8 = dt.float32, dt.bfloat16, dt.float8e4
    AF = mybir.ActivationFunctionType
    ALU = mybir.AluOpType
    DR = mybir.MatmulPerfMode.DoubleRow

    nc = bacc.Bacc("TRN2", target_bir_lowering=False, debug=False,
                   num_devices=N_CORES)

    # weights come pre-packed partition-major from the host so every SBUF
    # weight tile loads with one DMA of large contiguous per-partition lines
    xb = nc.dram_tensor("xb", [T, D], f32, kind="ExternalInput").ap()
    wqkv_pk = nc.dram_tensor("wqkv_pk", [P, 2 * KD, KD, P], f8,
                             kind="ExternalInput").ap()
    wv_pk = nc.dram_tensor("wv_pk", [P, KD, D], f8,
                           kind="ExternalInput").ap()
    bqkv = nc.dram_tensor("bqkv", [P, NF], f32, kind="ExternalInput").ap()
    wp_pk = nc.dram_tensor("wp_pk", [P, KD, D], f8,
                           kind="ExternalInput").ap()
    wfc1_pk = nc.dram_tensor("wfc1_pk", [P, NFF, KD, P], f8,
                             kind="ExternalInput").ap()
    bfc1 = nc.dram_tensor("bfc1", [P, NFF], f32, kind="ExternalInput").ap()
    wfc2_pk = nc.dram_tensor("wfc2_pk", [P, 2, NFF, 512], f8,
                             kind="ExternalInput").ap()
    bfc2 = nc.dram_tensor("bfc2", [P, D], f32, kind="ExternalInput").ap()
    xb2 = nc.dram_tensor("xb2", [T, D], f32, kind="ExternalInput").ap()
    out = nc.dram_tensor("out", [T, D], f32, kind="ExternalOutput").ap()

    with tile.TileContext(nc) as tc, ExitStack() as ctx:
        const = ctx.enter_context(tc.tile_pool(name="const", bufs=1))
        eps_t = const.tile([P, 1], f32)
        nc.vector.memset(eps_t, EPS)

        ident = const.tile([P, P], bf16)
        make_identity(nc, ident)
        ones_mat = const.tile([P, 64], bf16)
        nc.vector.memset(ones_mat, 1.0)

        bqkv_sb = const.tile([P, NF], f32)
        bfc1_sb = const.tile([P, NFF], f32)
        bfc2_bc = const.tile([P, D], f32)

        def ln_stats_norm(src_pool, stat_pool, x_t, norm_on_act=False):
            """x_t [P, D] f32 -> normalized bf16 tile (returned).

            Stats on DVE; optionally the normalize runs on ACT (Copy with a
            per-token scale after a DVE-side subtract) to split the LN load.
            """
            st = stat_pool.tile([P, 2, 6], f32, name="st")
            xr = x_t.rearrange("p (s q) -> p s q", s=2)
            nc.vector.bn_stats(out=st[:, 0, :], in_=xr[:, 0, :])
            nc.vector.bn_stats(out=st[:, 1, :], in_=xr[:, 1, :])
            mv = stat_pool.tile([P, 2], f32, name="mv")
            nc.vector.bn_aggr(out=mv, in_=st)
            rstd = stat_pool.tile([P, 1], f32, name="rstd")
            nc.scalar.activation(out=rstd, in_=mv[:, 1:2], func=AF.Sqrt,
                                 bias=eps_t)
            nc.vector.reciprocal(out=rstd, in_=rstd)
            h_bf = src_pool.tile([P, D], bf16, name="h_bf")
            if norm_on_act:
                nmr = stat_pool.tile([P, 1], f32, name="nmr")
                nc.vector.scalar_tensor_tensor(
                    out=nmr, in0=mv[:, 0:1], scalar=-1.0, in1=rstd,
                    op0=ALU.mult, op1=ALU.mult)
                nc.scalar.activation(out=h_bf, in_=x_t, func=AF.Identity,
                                     scale=rstd, bias=nmr)
            else:
                nc.vector.tensor_scalar(out=h_bf, in0=x_t,
                                        scalar1=mv[:, 0:1], scalar2=rstd,
                                        op0=ALU.subtract, op1=ALU.mult)
            return h_bf

        def ln_transpose(ps_pool, h_bf, dst_tile, it):
            # transpose via regular matmul against identity (out = h^T @ I):
            # all 8 chunks land in one 2-bank psum tile and leave through a
            # single 1024-wide scalar-engine copy.  Callers lag this one
            # tile behind the normalize so the PE never waits on it.
            pt = ps_pool.tile([P, T], f32, name="ps_mm")
            for kd in range(KD):
                nc.tensor.matmul(pt[:, kd * P:(kd + 1) * P],
                                 h_bf[:, kd * P:(kd + 1) * P], ident,
                                 start=True, stop=True)
            nc.scalar.copy(
                out=dst_tile[:, :, it * P:(it + 1) * P],
                in_=pt.rearrange("p (q t) -> p q t", q=KD))

        # Pool nesting is by lifetime (LIFO): x2 dies last, then w2a, oT, ...
        with tc.tile_pool(name="phX2", bufs=1) as phX2:
          x2 = phX2.tile([P, NT, D], f32)
          with tc.tile_pool(name="phOT", bufs=1) as phOT, \
               tc.tile_pool(name="phB", bufs=1) as phB, \
               tc.tile_pool(name="wpP", bufs=1) as wpP, \
               tc.tile_pool(name="xr", bufs=3) as xr_pool, \
               tc.tile_pool(name="stat2", bufs=6) as stat2_pool:
            oT = phOT.tile([P, KD, T], f8)
            qkvT = phB.tile([P, 2 * KD, T], bf16)
            wp_sb = wpP.tile([P, KD, D], f8)
            with tc.tile_pool(name="phC", bufs=1) as phC:
              v_tok = phC.tile([P, NT, D], f8)
              wv_sb = phC.tile([P, KD, D], f8)

              # ---------------- Phase A: LN1 -> hT, fused with the
              # attention prologue (pair-0 q/k chains and the first v pair
              # interleave into the LN loop once their token half is ready)
              # and phases B-D: software-pipelined attention over head
              # pairs. -------------
              with tc.tile_pool(name="phA", bufs=1) as phA:
                hT2 = [phA.tile([P, KD, T // 2], f8, name="hTa"),
                       phA.tile([P, KD, T // 2], f8, name="hTb")]
                with tc.tile_pool(name="xa", bufs=5) as xa_pool, \
                     tc.tile_pool(name="stat", bufs=6) as stat_pool, \
                     tc.tile_pool(name="wq", bufs=6) as wq_pool, \
                     tc.tile_pool(name="ptp", bufs=2) as pt_pool, \
                     tc.tile_pool(name="attn_sm", bufs=1) as sm_pool, \
                     tc.tile_pool(name="ps_sc", bufs=3,
                                  space="PSUM") as ps_sc, \
                     tc.tile_pool(name="ps_avp", bufs=1,
                                  space="PSUM") as ps_avp:

                    def emit_qkv_ft(ft):
                        # q/k feature tile: fp8 DoubleRow (256-contraction)
                        # chains; both token halves accumulate in one 2-bank
                        # psum tile, single 1024-wide descale+bias evict
                        w_t = wq_pool.tile([P, KD, P], f8, name="w_t")
                        nc.sync.dma_start(out=w_t, in_=wqkv_pk[:, ft])
                        ps = ps_sc.tile([P, T], f32, name="ps_mm")
                        for nh in range(2):
                            for k in range(0, KD, 2):
                                nc.tensor.matmul(
                                    ps[:, nh * 512:(nh + 1) * 512],
                                    w_t[:, k:k + 2, :],
                                    hT2[nh][:, k:k + 2, :],
                                    start=(k == 0), stop=(k == KD - 2),
                                    perf_mode=DR)
                        descale = 2.0 ** -10 if ft < KD else 2.0 ** -7
                        nc.vector.tensor_scalar(
                            out=qkvT[:, ft, :],
                            in0=ps, scalar1=descale,
                            scalar2=bqkv_sb[:, ft:ft + 1], op0=ALU.mult,
                            op1=ALU.add)

                    def emit_qkv_half(ft, nh):
                        # single token-half q/k chain for the phase-A
                        # prologue (only needs hT2[nh])
                        w_t = wq_pool.tile([P, KD, P], f8, name="w_t")
                        nc.sync.dma_start(out=w_t, in_=wqkv_pk[:, ft])
                        ps = ps_sc.tile([P, T], f32, name="ps_mm")
                        for k in range(0, KD, 2):
                            nc.tensor.matmul(
                                ps[:, 0:512],
                                w_t[:, k:k + 2, :],
                                hT2[nh][:, k:k + 2, :],
                                start=(k == 0), stop=(k == KD - 2),
                                perf_mode=DR)
                        descale = 2.0 ** -10 if ft < KD else 2.0 ** -7
                        nc.vector.tensor_scalar(
                            out=qkvT[:, ft, nh * 512:(nh + 1) * 512],
                            in0=ps[:, 0:512], scalar1=descale,
                            scalar2=bqkv_sb[:, ft:ft + 1], op0=ALU.mult,
                            op1=ALU.add)

                    def emit_v_pair(g, tt):
                        # v token-major for token tiles (tt, tt+1) of the
                        # 512-wide feature chunk g (v_bias is folded into
                        # the proj bias)
                        vsl = slice(g * 512, g * 512 + 512)
                        ps = ps_sc.tile([P, T], f32, name="ps_mm")
                        for ti in range(2):
                            hTh = hT2[(tt + ti) // 4]
                            to = ((tt + ti) % 4) * P
                            for k in range(0, KD, 2):
                                nc.tensor.matmul(
                                    ps[:, ti * 512:(ti + 1) * 512],
                                    hTh[:, k:k + 2, to:to + P],
                                    wv_sb[:, k:k + 2, vsl],
                                    start=(k == 0), stop=(k == KD - 2),
                                    perf_mode=DR)
                        nc.vector.tensor_scalar(
                            out=v_tok[:, tt:tt + 2, vsl], in0=ps,
                            scalar1=2.0 ** -7, scalar2=None, op0=ALU.mult)

                    # bulk weight prefetches ride the gpsimd queue so the
                    # x tiles stream unimpeded on the sync queue
                    nc.gpsimd.dma_start(out=wv_sb, in_=wv_pk)
                    nc.gpsimd.dma_start(out=bqkv_sb, in_=bqkv)

                    # LN1 over token tiles; transposes lag one tile so the
                    # PE issue stream never parks behind the DVE chain
                    h_prev = None
                    x_ts = {}
                    for it in range(2):
                        x_ts[it] = xa_pool.tile([P, D], f32, name="x_t")
                        nc.sync.dma_start(out=x_ts[it],
                                          in_=xb[it * P:(it + 1) * P, :])
                    for it in range(NT):
                        if it in x_ts:
                            x_t = x_ts.pop(it)
                        else:
                            x_t = xa_pool.tile([P, D], f32, name="x_t")
                            nc.sync.dma_start(out=x_t,
                                              in_=xb[it * P:(it + 1) * P, :])
                        h_cur = ln_stats_norm(xa_pool, stat_pool, x_t,
                                              norm_on_act=(it % 2 == 1))
                        if h_prev is not None:
                            ln_transpose(ps_sc, h_prev, hT2[(it - 1) // 4],
                                         (it - 1) % 4)
                        h_prev = h_cur
                        if it == 4:
                            emit_qkv_half(0, 0)
                            emit_qkv_half(KD, 0)
                        if it == 5:
                            emit_v_pair(0, 0)
                        if it == 6:
                            emit_v_pair(0, 2)
                        if it == 7:
                            emit_v_pair(0, 4)
                    ln_transpose(ps_sc, h_prev, hT2[1], 3)

                    state = {}

                    def av_tail(p):
                        # last AV chunk of pair p, deferred into pair p+1's
                        # first slot so it doesn't hold up that pair's scores
                        u4s, ps_av, PTl = state[p]
                        PTl = PTl()
                        for ch in range(2):
                            for j in range(HPF):
                                po = j * HD
                                hf = (HPF * p + j) * HD
                                nc.tensor.matmul(
                                    ps_av[po:po + HD,
                                          ch * 512:(ch + 1) * 512],
                                    v_tok[:, NT - 1, hf:hf + HD],
                                    PTl[j][:, ch * 512:(ch + 1) * 512],
                                    start=False, stop=True)

                    def finalize_pair(p):
                        # ones-matmuls broadcast this pair's column sums into
                        # 64 partitions per head of a 2-bank bc tile; recip +
                        # 1024-wide normalize eviction follow.
                        u4s, ps_av, _ = state[p]
                        bc = ps_sc.tile([P, T], f32, name="ps_mm")
                        for j in range(HPF):
                            po = j * HD
                            for ch in range(2):
                                nc.tensor.matmul(
                                    bc[po:po + HD, ch * 512:(ch + 1) * 512],
                                    ones_mat[:, 0:HD],
                                    u4s[j][:, ch * 512:(ch + 1) * 512],
                                    start=True, stop=True)
                        rec_sb = sm_pool.tile([P, T], f32, name="rec_sb")
                        nc.vector.reciprocal_approx_fast(out=rec_sb, in_=bc)
                        nc.vector.tensor_tensor(
                            out=oT[:, p, :], in0=ps_av, in1=rec_sb,
                            op=ALU.mult)

                    # prologue tail: pair 0's second token halves
                    emit_qkv_half(0, 1)
                    emit_qkv_half(KD, 1)

                    # filler chains per pair: qkv tiles for pair p+1, v
                    # chunks just-in-time for this group's AV consumption
                    fillers = {p: [] for p in range(H // HPF)}
                    fillers[0] = [lambda: emit_v_pair(0, 6)]
                    for p in range(H // HPF - 1):
                        fillers[p] += [
                            lambda ft=p + 1: emit_qkv_half(ft, 0),
                            lambda ft=KD + p + 1: emit_qkv_half(ft, 0),
                            lambda ft=p + 1: emit_qkv_half(ft, 1),
                            lambda ft=KD + p + 1: emit_qkv_half(ft, 1)]
                    fillers[2] += [lambda: emit_v_pair(1, 0)]
                    fillers[3] += [lambda: emit_v_pair(1, 2),
                                   lambda: emit_v_pair(1, 4)]
                    fillers[4] += [lambda: emit_v_pair(1, 6)]

                    for p in range(H // HPF):
                        ft_q = p
                        ft_k = KD + p
                        PT = {}
                        u4s = [sm_pool.tile([P, T], bf16, name=f"u4_{j}")
                               for j in range(HPF)]
                        ps_av = None
                        for mt in range(NT):
                            for j in range(HPF):
                                po = j * HD
                                ps_s = ps_sc.tile([P, T], f32, name="ps_mm")
                                for ch in range(2):
                                    nc.tensor.matmul(
                                        ps_s[:, ch * 512:(ch + 1) * 512],
                                        qkvT[po:po + HD, ft_k,
                                             mt * P:(mt + 1) * P],
                                        qkvT[po:po + HD, ft_q,
                                             ch * 512:(ch + 1) * 512],
                                        start=True, stop=True)
                                # per-chunk PT ring (4 deep per head) so the
                                # next pair's exp never waits on this pair's
                                # AV tail; 1024-wide exp amortizes ACT cost
                                PT[(j, mt)] = pt_pool.tile(
                                    [P, T], bf16, name=f"PT{j}", bufs=4)
                                nc.scalar.activation(
                                    out=PT[(j, mt)], in_=ps_s,
                                    func=AF.Exp)
                                if mt == 1:
                                    nc.vector.tensor_tensor(
                                        out=u4s[j], in0=PT[(j, 0)],
                                        in1=PT[(j, 1)], op=ALU.add)
                                elif mt > 1:
                                    nc.vector.tensor_tensor(
                                        out=u4s[j], in0=u4s[j],
                                        in1=PT[(j, mt)], op=ALU.add)
                            if mt == 0 and p > 0:
                                av_tail(p - 1)
                                finalize_pair(p - 1)
                            if mt == 5 and p == 4:
                                # prefetch the proj weights on the
                                # otherwise-idle sync queue mid-attention
                                nc.sync.dma_start(out=wp_sb, in_=wp_pk)
                            if 1 <= mt <= len(fillers[p]):
                                fillers[p][mt - 1]()
                            elif mt == 7 and len(fillers[p]) == 7:
                                fillers[p][6]()
                            if mt > 0:
                                if ps_av is None:
                                    ps_av = ps_avp.tile([P, T], f32,
                                                        name="ps_av")
                                    state[p] = (
                                        u4s, ps_av,
                                        lambda PT=PT: [PT[(j, NT - 1)]
                                                       for j in range(HPF)])
                                for ch in range(2):
                                    for j in range(HPF):
                                        po = j * HD
                                        hf = (HPF * p + j) * HD
                                        nc.tensor.matmul(
                                            ps_av[po:po + HD,
                                                  ch * 512:(ch + 1) * 512],
                                            v_tok[:, mt - 1, hf:hf + HD],
                                            PT[(j, mt - 1)][
                                                :, ch * 512:(ch + 1) * 512],
                                            start=(mt == 1), stop=False)
                    av_tail(H // HPF - 1)
                    finalize_pair(H // HPF - 1)
                    # preload the sqrt ACT table set while the scalar
                    # engine is idle so LN2's first rstd isn't behind a
                    # 1.3us table switch
                    warm = stat2_pool.tile([P, 1], f32, name="warm")
                    nc.scalar.activation(out=warm, in_=eps_t, func=AF.Sqrt,
                                         bias=eps_t)

            # -------- Phases E2+F fused: proj + residual -> x2, LN2 and
            # the first fc1 half-chains pipelined per token tile (qkvT/v/o
            # freed when the enclosing pools close).  LN2 transposes lag one
            # tile behind the proj/normalize chain so the PE stream never
            # parks on a cross-engine dependency. ------
            with tc.tile_pool(name="phG2", bufs=1) as phG2:
              aT = phG2.tile([P, NFF, T], f8)
              w2a = phG2.tile([P, NFF, 512], f8, name="w2a")
              nc.gpsimd.dma_start(out=bfc1_sb, in_=bfc1)
              nc.gpsimd.dma_start(out=bfc2_bc, in_=bfc2)
              nc.gpsimd.dma_start(out=w2a, in_=wfc2_pk[:, 0])
              with tc.tile_pool(name="phF", bufs=1) as phF:
                h2T2 = [phF.tile([P, KD, T // 2], f8, name="h2Ta"),
                        phF.tile([P, KD, T // 2], f8, name="h2Tb")]
                NEARLY = 8

                with tc.tile_pool(name="w1h", bufs=4) as w1h_pool, \
                     tc.tile_pool(name="ps_e", bufs=3,
                                  space="PSUM") as ps_e, \
                     tc.tile_pool(name="ps_f1h", bufs=2,
                                  space="PSUM") as ps_f1h:

                    def fc1_half(w_pool, ps_pool, ff, nh):
                        w_t = w_pool.tile([P, KD, P], f8, name="w1_t")
                        nc.sync.dma_start(out=w_t, in_=wfc1_pk[:, ff])
                        ps = ps_pool.tile([P, 512], f32, name="ps_half",
                                          bufs=2)
                        for k in range(0, KD, 2):
                            nc.tensor.matmul(
                                ps, w_t[:, k:k + 2, :],
                                h2T2[nh][:, k:k + 2, :],
                                start=(k == 0), stop=(k == KD - 2),
                                perf_mode=DR)
                        nc.scalar.activation(
                            out=aT[:, ff, nh * 512:(nh + 1) * 512],
                            in_=ps, func=AF.Gelu, scale=2.0 ** -7,
                            bias=bfc1_sb[:, ff:ff + 1])

                    # proj sweep first: with only the cheap evictions on
                    # DVE the proj stream runs PE-bound back-to-back; the
                    # LN2 stats chains (the expensive DVE work) follow in a
                    # second pipelined sweep so they never gate the proj
                    # PSUM ring
                    x_rs = {}
                    for tt in range(3):
                        x_rs[tt] = xr_pool.tile([P, D], f32, name="x_r")
                        nc.sync.dma_start(out=x_rs[tt],
                                          in_=xb2[tt * P:(tt + 1) * P, :])
                    for tt in range(NT):
                        if tt in x_rs:
                            x_r = x_rs.pop(tt)
                        else:
                            x_r = xr_pool.tile([P, D], f32, name="x_r")
                            nc.sync.dma_start(out=x_r,
                                              in_=xb2[tt * P:(tt + 1) * P, :])
                        ps = ps_e.tile([P, T], f32, name="ps_mm")
                        for dh in range(2):
                            for k in range(0, KD, 2):
                                nc.tensor.matmul(
                                    ps[:, dh * 512:(dh + 1) * 512],
                                    oT[:, k:k + 2, tt * P:(tt + 1) * P],
                                    wp_sb[:, k:k + 2,
                                          dh * 512:(dh + 1) * 512],
                                    start=(k == 0), stop=(k == KD - 2),
                                    perf_mode=DR)
                        # x_r already carries x + proj bias (host-folded),
                        # so proj evict + bias + residual is one DVE op
                        nc.vector.scalar_tensor_tensor(
                            out=x2[:, tt, :], in0=ps, scalar=2.0 ** -7,
                            in1=x_r, op0=ALU.mult, op1=ALU.add)
                    h2_prev = None
                    for tt in range(NT):
                        h2_cur = ln_stats_norm(xr_pool, stat2_pool,
                                               x2[:, tt, :])
                        if h2_prev is not None:
                            ln_transpose(ps_e, h2_prev, h2T2[(tt - 1) // 4],
                                         (tt - 1) % 4)
                        h2_prev = h2_cur
                        if tt >= 4:
                            fc1_half(w1h_pool, ps_f1h, 2 * (tt - 4), 0)
                            fc1_half(w1h_pool, ps_f1h, 2 * (tt - 4) + 1, 0)
                    ln_transpose(ps_e, h2_prev, h2T2[1], 3)

                # ------- Phase G: the rest of fc1 -------
                with tc.tile_pool(name="w1", bufs=6) as w1_pool, \
                     tc.tile_pool(name="ps_f1", bufs=3,
                                  space="PSUM") as ps_f1:
                    for ff in range(NFF):
                        if ff < NEARLY:
                            fc1_half(w1_pool, ps_f1, ff, 1)
                            continue
                        w_t = w1_pool.tile([P, KD, P], f8, name="w1_t")
                        nc.sync.dma_start(out=w_t, in_=wfc1_pk[:, ff])
                        ps = ps_f1.tile([P, T], f32, name="ps_mm")
                        for nh in range(2):
                            for k in range(0, KD, 2):
                                nc.tensor.matmul(
                                    ps[:, nh * 512:(nh + 1) * 512],
                                    w_t[:, k:k + 2, :],
                                    h2T2[nh][:, k:k + 2, :],
                                    start=(k == 0), stop=(k == KD - 2),
                                    perf_mode=DR)
                        nc.scalar.activation(
                            out=aT[:, ff, :],
                            in_=ps, func=AF.Gelu, scale=2.0 ** -7,
                            bias=bfc1_sb[:, ff:ff + 1])

              # ------- Phase H: fc2 + residual -> out (h2T2 freed) -------
              with tc.tile_pool(name="w2b", bufs=1) as w2b_pool, \
                   tc.tile_pool(name="yb", bufs=3) as y_pool, \
                   tc.tile_pool(name="ps_f2", bufs=4,
                                space="PSUM") as ps_f2:
                w2b = w2b_pool.tile([P, NFF, 512], f8, name="w2b")
                nc.gpsimd.dma_start(out=w2b, in_=wfc2_pk[:, 1])
                for dh, w2_t in ((0, w2a), (1, w2b)):
                    sl = slice(dh * 512, (dh + 1) * 512)
                    for tt in range(NT):
                        ps = ps_f2.tile([P, 512], f32, name="ps_mm")
                        for k in range(0, NFF, 2):
                            nc.tensor.matmul(
                                ps, aT[:, k:k + 2, tt * P:(tt + 1) * P],
                                w2_t[:, k:k + 2, :],
                                start=(k == 0), stop=(k == NFF - 2),
                                perf_mode=DR)
                        y_sb = y_pool.tile([P, 512], f32, name="y_sb")
                        nc.vector.scalar_tensor_tensor(
                            out=y_sb, in0=ps, scalar=2.0 ** -8,
                            in1=bfc2_bc[:, sl], op0=ALU.mult, op1=ALU.add)
                        nc.vector.tensor_tensor(
                            out=y_sb, in0=y_sb,
                            in1=x2[:, tt, sl], op=ALU.add)
                        dma_eng = nc.sync if (tt + dh) % 2 == 0 \
                            else nc.gpsimd
                        dma_eng.dma_start(
                            out=out[tt * P:(tt + 1) * P, sl],
                            in_=y_sb)

    nc.compile()
    return nc


def _prep_host_inputs(x, ln1_g, ln1_b, ln2_g, ln2_b, qkv_w, q_bias, v_bias,
                      proj_w, proj_b, fc1_w, fc1_b, fc2_w, fc2_b):
    f32 = np.float32
    x = np.asarray(x, f32)
    ln1_g = np.asarray(ln1_g, f32)
    ln1_b = np.asarray(ln1_b, f32)
    ln2_g = np.asarray(ln2_g, f32)
    ln2_b = np.asarray(ln2_b, f32)
    qkv_w = np.asarray(qkv_w, f32)
    q_bias = np.asarray(q_bias, f32)
    v_bias = np.asarray(v_bias, f32)
    proj_w = np.asarray(proj_w, f32)
    proj_b = np.asarray(proj_b, f32)
    fc1_w = np.asarray(fc1_w, f32)
    fc1_b = np.asarray(fc1_b, f32)
    fc2_w = np.asarray(fc2_w, f32)
    fc2_b = np.asarray(fc2_b, f32)

    scale = HD ** (-0.5)
    # v_bias is a constant per-feature shift of o (softmax rows sum to 1),
    # so it folds into the proj bias: proj_b += proj_w @ v_bias.  The v part
    # of the qkv bias used on-chip is qkv_w_v @ ln1_b only.
    qkv_bias = np.concatenate(
        [q_bias, np.zeros_like(v_bias), np.zeros_like(v_bias)])
    wqkv = qkv_w * ln1_g[None, :]
    bqkv = qkv_w @ ln1_b + qkv_bias
    wqkv = wqkv.copy()
    wqkv[:D] *= scale
    bqkv[:D] *= scale
    proj_b = proj_b + proj_w @ (qkv_w[2 * D:] @ ln1_b + v_bias)

    wfc1 = fc1_w * ln2_g[None, :]
    bfc1 = fc1_w @ ln2_b + fc1_b

    # qkv/fc1/fc2 weights are cast to fp8e4m3 with power-of-two pre-scales
    # (q rows carry an extra 1/8 from the attention scale fold); the kernel
    # multiplies the matching 2^-s back in during psum eviction.  All weight
    # matrices are repacked partition-major so each SBUF tile loads with one
    # DMA of large contiguous per-partition lines.
    f8 = ml_dtypes.float8_e4m3
    KD_, NFF_, NF_ = D // 128, FF // 128, 3 * D // 128
    qkv_colscale = np.concatenate(
        [np.full(D, 2.0 ** 10, f32), np.full(2 * D, 2.0 ** 7, f32)])
    wqkvT8 = (wqkv.T * qkv_colscale[None, :]).astype(f8)
    wqkv_pk = wqkvT8[:, :2 * D].reshape(KD_, 128, 2 * KD_, 128) \
        .transpose(1, 2, 0, 3)
    wv_pk = wqkvT8[:, 2 * D:].reshape(KD_, 128, D).transpose(1, 0, 2)
    wp_pk = (proj_w.T * 2.0 ** 7).astype(f8) \
        .reshape(KD_, 128, D).transpose(1, 0, 2)
    wfc1_pk = (wfc1.T * 2.0 ** 7).astype(f8) \
        .reshape(KD_, 128, NFF_, 128).transpose(1, 2, 0, 3)
    wfc2_pk = (fc2_w.T * 2.0 ** 8).astype(f8) \
        .reshape(NFF_, 128, 2, 512).transpose(1, 2, 0, 3)
    shared = {
        "wqkv_pk": np.ascontiguousarray(wqkv_pk),
        "wv_pk": np.ascontiguousarray(wv_pk),
        "bqkv": np.ascontiguousarray(bqkv.reshape(NF_, 128).T, f32),
        "wp_pk": np.ascontiguousarray(wp_pk),
        "wfc1_pk": np.ascontiguousarray(wfc1_pk),
        "bfc1": np.ascontiguousarray(bfc1.reshape(NFF_, 128).T, f32),
        "wfc2_pk": np.ascontiguousarray(wfc2_pk),
        "bfc2": np.ascontiguousarray(
            np.broadcast_to(fc2_b, (128, D)), f32),
    }
    # residual stream with the proj bias pre-added (saves an on-chip add)
    x2r = x + proj_b[None, None, :]
    in_maps = [dict(shared, xb=np.ascontiguousarray(x[i]),
                    xb2=np.ascontiguousarray(x2r[i], f32))
               for i in range(N_CORES)]
    return in_maps


def kernel(**inputs):
    from concourse.bass_utils import run_bass_kernel_spmd

    if "nc" not in _CACHE:
        _CACHE["nc"] = _build_nc()
    nc = _CACHE["nc"]
    in_maps = _prep_host_inputs(**inputs)
    res = run_bass_kernel_spmd(nc, in_maps, core_ids=list(range(N_CORES)),
                               trace=False)
    return np.stack([res.results[i]["out"] for i in range(N_CORES)], axis=0)


if __name__ == "__main__":
    rng = np.random.default_rng(0)
    ins = {
        "x": rng.standard_normal((B, T, D)).astype(np.float32),
        "ln1_g": np.ones(D, np.float32), "ln1_b": np.zeros(D, np.float32),
        "ln2_g": np.ones(D, np.float32), "ln2_b": np.zeros(D, np.float32),
        "qkv_w": (rng.uniform(-1, 1, (3 * D, D)) / 32).astype(np.float32),
        "q_bias": np.zeros(D, np.float32), "v_bias": np.zeros(D, np.float32),
        "proj_w": (rng.uniform(-1, 1, (D, D)) / 32).astype(np.float32),
        "proj_b": np.zeros(D, np.float32),
        "fc1_w": (rng.uniform(-1, 1, (FF, D)) / 32).astype(np.float32),
        "fc1_b": np.zeros(FF, np.float32),
        "fc2_w": (rng.uniform(-1, 1, (D, FF)) / 64).astype(np.float32),
        "fc2_b": np.zeros(D, np.float32),
    }
    y = kernel(**ins)
    print("out", y.shape, y.dtype, np.abs(y).max())


# revision 26
# speedup vs baseline: 1.0189x; 1.0051x over previous
"""Trainium2 Bass kernel for a dense transformer block.

Strategy: data-parallel over batch (8 batch elems -> 8 cores, no collectives).
Per core: x[1024, 1024] through LN1 -> qkv -> attention -> proj(+res) -> LN2 ->
fc1 -> gelu -> fc2(+res). Matmuls in fp8 DoubleRow where possible with fp32
PSUM accumulation.  LayerNorm gamma/beta are folded into the following matmul's
weights/bias on the host, and the attention scale 1/sqrt(hd) is folded into the
q-part of the qkv weights.

Attention uses a transposed-scores layout: S^T[m, n] tiles come straight out of
the PE with keys (m) on partitions, exp() is applied on eviction (no max
subtraction needed: inputs are layernormed, |scores| is O(1)), and the P^T @ v
matmul contracts m on partitions.  The two heads of a pair live in partition
rows 0-63 / 64-127 so their score / AV matmuls dual-issue in the PE array.
The attention inner loop is software-pipelined at m-chunk granularity with the
exp() evictions on the scalar engine as the pacing resource; softmax column
sums are split between DVE (head 0) and GpSimd (head 1) so no single engine
outruns the exp stream.  PSUM is partitioned into a dedicated AV accumulator
(2 banks) and a 6-bank rotation shared by score tiles, filler qkv/v chains and
the softmax-denominator broadcast.  proj -> LN2 -> fc1 are fused into one
pipelined loop (LN2 transposes lag one tile so the PE never waits on the
DVE normalize chain), and LN normalizes alternate between the scalar and
vector engines.
"""

import numpy as np
import ml_dtypes

B = 8
T = 1024
D = 1024
H = 16
HD = D // H
FF = 4096
EPS = 1e-5
P = 128
N_CORES = 8

NT = T // P      # 8 token tiles
KD = D // P      # 8 contraction chunks over d
NF = 3 * D // P  # 24 qkv feature tiles
NFF = FF // P    # 32 ff feature tiles
HPF = P // HD    # 2 heads per 128-feature tile

_CACHE = {}


def _build_nc():
    from contextlib import ExitStack

    import concourse.bass as bass
    import concourse.mybir as mybir
    import concourse.tile as tile
    from concourse import bacc
    from concourse.masks import make_identity

    dt = mybir.dt
    f32, bf16, f8 = dt.float32, dt.bfloat16, dt.float8e4
    AF = mybir.ActivationFunctionType
    ALU = mybir.AluOpType
    DR = mybir.MatmulPerfMode.DoubleRow

    nc = bacc.Bacc("TRN2", target_bir_lowering=False, debug=False,
                   num_devices=N_CORES)

    # weights come pre-packed partition-major from the host so every SBUF
    # weight tile loads with one DMA of large contiguous per-partition lines
    xb = nc.dram_tensor("xb", [T, D], f32, kind="ExternalInput").ap()
    wqkv_pk = nc.dram_tensor("wqkv_pk", [P, 2 * KD, KD, P], f8,
                             kind="ExternalInput").ap()
    wv_pk = nc.dram_tensor("wv_pk", [P, KD, D], f8,
                           kind="ExternalInput").ap()
    bqkv = nc.dram_tensor("bqkv", [P, NF], f32, kind="ExternalInput").ap()
    wp_pk = nc.dram_tensor("wp_pk", [P, KD, D], f8,
                           kind="ExternalInput").ap()
    wfc1_pk = nc.dram_tensor("wfc1_pk", [P, NFF, KD, P], f8,
                             kind="ExternalInput").ap()
    bfc1 = nc.dram_tensor("bfc1", [P, NFF], f32, kind="ExternalInput").ap()
    wfc2_pk = nc.dram_tensor("wfc2_pk", [P, 2, NFF, 512], f8,
                             kind="ExternalInput").ap()
    bfc2 = nc.dram_tensor("bfc2", [P, D], f32, kind="ExternalInput").ap()
    xb2 = nc.dram_tensor("xb2", [T, D], f32, kind="ExternalInput").ap()
    out = nc.dram_tensor("out", [T, D], f32, kind="ExternalOutput").ap()

    with tile.TileContext(nc) as tc, ExitStack() as ctx:
        const = ctx.enter_context(tc.tile_pool(name="const", bufs=1))
        eps_t = const.tile([P, 1], f32)
        nc.vector.memset(eps_t, EPS)

        ident = const.tile([P, P], bf16)
        make_identity(nc, ident)
        ones_mat = const.tile([P, 64], bf16)
        nc.vector.memset(ones_mat, 1.0)

        bqkv_sb = const.tile([P, NF], f32)
        bfc1_sb = const.tile([P, NFF], f32)
        bfc2_bc = const.tile([P, D], f32)

        def ln_stats_norm(src_pool, stat_pool, x_t, norm_on_act=False):
            """x_t [P, D] f32 -> normalized bf16 tile (returned).

            Stats on DVE; optionally the normalize runs on ACT (Copy with a
            per-token scale after a DVE-side subtract) to split the LN load.
            """
            st = stat_pool.tile([P, 2, 6], f32, name="st")
            xr = x_t.rearrange("p (s q) -> p s q", s=2)
            nc.vector.bn_stats(out=st[:, 0, :], in_=xr[:, 0, :])
            nc.vector.bn_stats(out=st[:, 1, :], in_=xr[:, 1, :])
            mv = stat_pool.tile([P, 2], f32, name="mv")
            nc.vector.bn_aggr(out=mv, in_=st)
            rstd = stat_pool.tile([P, 1], f32, name="rstd")
            nc.scalar.activation(out=rstd, in_=mv[:, 1:2], func=AF.Sqrt,
                                 bias=eps_t)
            nc.vector.reciprocal(out=rstd, in_=rstd)
            h_bf = src_pool.tile([P, D], bf16, name="h_bf")
            if norm_on_act:
                nmr = stat_pool.tile([P, 1], f32, name="nmr")
                nc.vector.scalar_tensor_tensor(
                    out=nmr, in0=mv[:, 0:1], scalar=-1.0, in1=rstd,
                    op0=ALU.mult, op1=ALU.mult)
                nc.scalar.activation(out=h_bf, in_=x_t, func=AF.Identity,
                                     scale=rstd, bias=nmr)
            else:
                nc.vector.tensor_scalar(out=h_bf, in0=x_t,
                                        scalar1=mv[:, 0:1], scalar2=rstd,
                                        op0=ALU.subtract, op1=ALU.mult)
            return h_bf

        def ln_transpose(ps_pool, h_bf, dst_tile, it):
            # transpose via regular matmul against identity (out = h^T @ I):
            # all 8 chunks land in one 2-bank psum tile and leave through a
            # single 1024-wide scalar-engine copy.  Callers lag this one
            # tile behind the normalize so the PE never waits on it.
            pt = ps_pool.tile([P, T], f32, name="ps_mm")
            for kd in range(KD):
                nc.tensor.matmul(pt[:, kd * P:(kd + 1) * P],
                                 h_bf[:, kd * P:(kd + 1) * P], ident,
                                 start=True, stop=True)
            nc.scalar.copy(
                out=dst_tile[:, :, it * P:(it + 1) * P],
                in_=pt.rearrange("p (q t) -> p q t", q=KD))

        # Pool nesting is by lifetime (LIFO): x2 dies last, then w2a, oT, ...
        with tc.tile_pool(name="phX2", bufs=1) as phX2:
          x2 = phX2.tile([P, NT, D], f32)
          with tc.tile_pool(name="phOT", bufs=1) as phOT, \
               tc.tile_pool(name="phB", bufs=1) as phB, \
               tc.tile_pool(name="wpP", bufs=1) as wpP, \
               tc.tile_pool(name="xr", bufs=3) as xr_pool, \
               tc.tile_pool(name="stat2", bufs=6) as stat2_pool:
            oT = phOT.tile([P, KD, T], f8)
            qkvT = phB.tile([P, 2 * KD, T], bf16)
            wp_sb = wpP.tile([P, KD, D], f8)
            with tc.tile_pool(name="phC", bufs=1) as phC:
              v_tok = phC.tile([P, NT, D], f8)
              wv_sb = phC.tile([P, KD, D], f8)

              # ---------------- Phase A: LN1 -> hT, fused with the
              # attention prologue (pair-0 q/k chains and the first v pair
              # interleave into the LN loop once their token half is ready)
              # and phases B-D: software-pipelined attention over head
              # pairs. -------------
              with tc.tile_pool(name="phA", bufs=1) as phA:
                hT2 = [phA.tile([P, KD, T // 2], f8, name="hTa"),
                       phA.tile([P, KD, T // 2], f8, name="hTb")]
                with tc.tile_pool(name="xa", bufs=8) as xa_pool, \
                     tc.tile_pool(name="stat", bufs=6) as stat_pool, \
                     tc.tile_pool(name="wq", bufs=6) as wq_pool, \
                     tc.tile_pool(name="ptp", bufs=2) as pt_pool, \
                     tc.tile_pool(name="attn_sm", bufs=1) as sm_pool, \
                     tc.tile_pool(name="ps_sc", bufs=3,
                                  space="PSUM") as ps_sc, \
                     tc.tile_pool(name="ps_avp", bufs=1,
                                  space="PSUM") as ps_avp:

                    def emit_qkv_ft(ft):
                        # q/k feature tile: fp8 DoubleRow (256-contraction)
                        # chains; both token halves accumulate in one 2-bank
                        # psum tile, single 1024-wide descale+bias evict
                        w_t = wq_pool.tile([P, KD, P], f8, name="w_t")
                        nc.sync.dma_start(out=w_t, in_=wqkv_pk[:, ft])
                        ps = ps_sc.tile([P, T], f32, name="ps_mm")
                        for nh in range(2):
                            for k in range(0, KD, 2):
                                nc.tensor.matmul(
                                    ps[:, nh * 512:(nh + 1) * 512],
                                    w_t[:, k:k + 2, :],
                                    hT2[nh][:, k:k + 2, :],
                                    start=(k == 0), stop=(k == KD - 2),
                                    perf_mode=DR)
                        descale = 2.0 ** -10 if ft < KD else 2.0 ** -7
                        nc.vector.tensor_scalar(
                            out=qkvT[:, ft, :],
                            in0=ps, scalar1=descale,
                            scalar2=bqkv_sb[:, ft:ft + 1], op0=ALU.mult,
                            op1=ALU.add)

                    def emit_qkv_half(ft, nh):
                        # single token-half q/k chain for the phase-A
                        # prologue (only needs hT2[nh])
                        w_t = wq_pool.tile([P, KD, P], f8, name="w_t")
                        nc.sync.dma_start(out=w_t, in_=wqkv_pk[:, ft])
                        ps = ps_sc.tile([P, T], f32, name="ps_mm")
                        for k in range(0, KD, 2):
                            nc.tensor.matmul(
                                ps[:, 0:512],
                                w_t[:, k:k + 2, :],
                                hT2[nh][:, k:k + 2, :],
                                start=(k == 0), stop=(k == KD - 2),
                                perf_mode=DR)
                        descale = 2.0 ** -10 if ft < KD else 2.0 ** -7
                        nc.vector.tensor_scalar(
                            out=qkvT[:, ft, nh * 512:(nh + 1) * 512],
                            in0=ps[:, 0:512], scalar1=descale,
                            scalar2=bqkv_sb[:, ft:ft + 1], op0=ALU.mult,
                            op1=ALU.add)

                    def emit_v_pair(g, tt):
                        # v token-major for token tiles (tt, tt+1) of the
                        # 512-wide feature chunk g (v_bias is folded into
                        # the proj bias)
                        vsl = slice(g * 512, g * 512 + 512)
                        ps = ps_sc.tile([P, T], f32, name="ps_mm")
                        for ti in range(2):
                            hTh = hT2[(tt + ti) // 4]
                            to = ((tt + ti) % 4) * P
                            for k in range(0, KD, 2):
                                nc.tensor.matmul(
                                    ps[:, ti * 512:(ti + 1) * 512],
                                    hTh[:, k:k + 2, to:to + P],
                                    wv_sb[:, k:k + 2, vsl],
                                    start=(k == 0), stop=(k == KD - 2),
                                    perf_mode=DR)
                        nc.vector.tensor_scalar(
                            out=v_tok[:, tt:tt + 2, vsl], in0=ps,
                            scalar1=2.0 ** -7, scalar2=None, op0=ALU.mult)

                    # bulk weight prefetches ride the gpsimd queue so the
                    # x tiles stream unimpeded on the sync queue
                    nc.gpsimd.dma_start(out=wv_sb, in_=wv_pk)
                    nc.gpsimd.dma_start(out=bqkv_sb, in_=bqkv)

                    # LN1 over token tiles; transposes lag one tile so the
                    # PE issue stream never parks behind the DVE chain
                    h_prev = None
                    x_ts = {}
                    for it in range(2):
                        x_ts[it] = xa_pool.tile([P, D], f32, name="x_t")
                        nc.sync.dma_start(out=x_ts[it],
                                          in_=xb[it * P:(it + 1) * P, :])
                    for it in range(NT):
                        if it in x_ts:
                            x_t = x_ts.pop(it)
                        else:
                            x_t = xa_pool.tile([P, D], f32, name="x_t")
                            nc.sync.dma_start(out=x_t,
                                              in_=xb[it * P:(it + 1) * P, :])
                        h_cur = ln_stats_norm(xa_pool, stat_pool, x_t,
                                              norm_on_act=(it % 2 == 1))
                        if h_prev is not None:
                            ln_transpose(ps_sc, h_prev, hT2[(it - 1) // 4],
                                         (it - 1) % 4)
                        h_prev = h_cur
                        if it == 4:
                            emit_qkv_half(0, 0)
                            emit_qkv_half(KD, 0)
                        if it == 5:
                            emit_v_pair(0, 0)
                        if it == 6:
                            emit_v_pair(0, 2)
                        if it == 7:
                            emit_v_pair(0, 4)
                    ln_transpose(ps_sc, h_prev, hT2[1], 3)

                    state = {}

                    def av_tail(p):
                        # last AV chunk of pair p, deferred into pair p+1's
                        # first slot so it doesn't hold up that pair's scores
                        u4s, ps_av, PTl = state[p]
                        PTl = PTl()
                        for ch in range(2):
                            for j in range(HPF):
                                po = j * HD
                                hf = (HPF * p + j) * HD
                                nc.tensor.matmul(
                                    ps_av[po:po + HD,
                                          ch * 512:(ch + 1) * 512],
                                    v_tok[:, NT - 1, hf:hf + HD],
                                    PTl[j][:, ch * 512:(ch + 1) * 512],
                                    start=False, stop=True)

                    def finalize_pair(p):
                        # ones-matmuls broadcast this pair's column sums into
                        # 64 partitions per head of a 2-bank bc tile; recip +
                        # 1024-wide normalize eviction follow.
                        u4s, ps_av, _ = state[p]
                        bc = ps_sc.tile([P, T], f32, name="ps_mm")
                        for j in range(HPF):
                            po = j * HD
                            for ch in range(2):
                                nc.tensor.matmul(
                                    bc[po:po + HD, ch * 512:(ch + 1) * 512],
                                    ones_mat[:, 0:HD],
                                    u4s[j][:, ch * 512:(ch + 1) * 512],
                                    start=True, stop=True)
                        rec_sb = sm_pool.tile([P, T], f32, name="rec_sb")
                        nc.vector.reciprocal_approx_fast(out=rec_sb, in_=bc)
                        nc.vector.tensor_tensor(
                            out=oT[:, p, :], in0=ps_av, in1=rec_sb,
                            op=ALU.mult)

                    # prologue tail: pair 0's second token halves
                    emit_qkv_half(0, 1)
                    emit_qkv_half(KD, 1)

                    # filler chains per pair: qkv tiles for pair p+1, v
                    # chunks just-in-time for this group's AV consumption
                    fillers = {p: [] for p in range(H // HPF)}
                    fillers[0] = [lambda: emit_v_pair(0, 6)]
                    for p in range(H // HPF - 1):
                        fillers[p] += [
                            lambda ft=p + 1: emit_qkv_half(ft, 0),
                            lambda ft=KD + p + 1: emit_qkv_half(ft, 0),
                            lambda ft=p + 1: emit_qkv_half(ft, 1),
                            lambda ft=KD + p + 1: emit_qkv_half(ft, 1)]
                    fillers[2] += [lambda: emit_v_pair(1, 0)]
                    fillers[3] += [lambda: emit_v_pair(1, 2),
                                   lambda: emit_v_pair(1, 4)]
                    fillers[4] += [lambda: emit_v_pair(1, 6)]

                    for p in range(H // HPF):
                        ft_q = p
                        ft_k = KD + p
                        PT = {}
                        u4s = [sm_pool.tile([P, T], bf16, name=f"u4_{j}")
                               for j in range(HPF)]
                        ps_av = None
                        for mt in range(NT):
                            for j in range(HPF):
                                po = j * HD
                                ps_s = ps_sc.tile([P, T], f32, name="ps_mm")
                                for ch in range(2):
                                    nc.tensor.matmul(
                                        ps_s[:, ch * 512:(ch + 1) * 512],
                                        qkvT[po:po + HD, ft_k,
                                             mt * P:(mt + 1) * P],
                                        qkvT[po:po + HD, ft_q,
                                             ch * 512:(ch + 1) * 512],
                                        start=True, stop=True)
                                # per-chunk PT ring (4 deep per head) so the
                                # next pair's exp never waits on this pair's
                                # AV tail; 1024-wide exp amortizes ACT cost
                                PT[(j, mt)] = pt_pool.tile(
                                    [P, T], bf16, name=f"PT{j}", bufs=4)
                                nc.scalar.activation(
                                    out=PT[(j, mt)], in_=ps_s,
                                    func=AF.Exp)
                                if mt == 1:
                                    nc.vector.tensor_tensor(
                                        out=u4s[j], in0=PT[(j, 0)],
                                        in1=PT[(j, 1)], op=ALU.add)
                                elif mt > 1:
                                    nc.vector.tensor_tensor(
                                        out=u4s[j], in0=u4s[j],
                                        in1=PT[(j, mt)], op=ALU.add)
                            if mt == 0 and p > 0:
                                av_tail(p - 1)
                                finalize_pair(p - 1)
                            if mt == 5 and p == 4:
                                # prefetch the proj weights on the
                                # otherwise-idle sync queue mid-attention
                                nc.sync.dma_start(out=wp_sb, in_=wp_pk)
                            if 1 <= mt <= len(fillers[p]):
                                fillers[p][mt - 1]()
                            elif mt == 7 and len(fillers[p]) == 7:
                                fillers[p][6]()
                            if mt > 0:
                                if ps_av is None:
                                    ps_av = ps_avp.tile([P, T], f32,
                                                        name="ps_av")
                                    state[p] = (
                                        u4s, ps_av,
                                        lambda PT=PT: [PT[(j, NT - 1)]
                                                       for j in range(HPF)])
                                for ch in range(2):
                                    for j in range(HPF):
                                        po = j * HD
                                        hf = (HPF * p + j) * HD
                                        nc.tensor.matmul(
                                            ps_av[po:po + HD,
                                                  ch * 512:(ch + 1) * 512],
                                            v_tok[:, mt - 1, hf:hf + HD],
                                            PT[(j, mt - 1)][
                                                :, ch * 512:(ch + 1) * 512],
                                            start=(mt == 1), stop=False)
                    av_tail(H // HPF - 1)
                    finalize_pair(H // HPF - 1)
                    # preload the sqrt ACT table set while the scalar
                    # engine is idle so LN2's first rstd isn't behind a
                    # 1.3us table switch
                    warm = stat2_pool.tile([P, 1], f32, name="warm")
                    nc.scalar.activation(out=warm, in_=eps_t, func=AF.Sqrt,
                                         bias=eps_t)

            # -------- Phases E2+F fused: proj + residual -> x2, LN2 and
            # the first fc1 half-chains pipelined per token tile (qkvT/v/o
            # freed when the enclosing pools close).  LN2 transposes lag one
            # tile behind the proj/normalize chain so the PE stream never
            # parks on a cross-engine dependency. ------
            with tc.tile_pool(name="phG2", bufs=1) as phG2:
              aT = phG2.tile([P, NFF, T], f8)
              w2a = phG2.tile([P, NFF, 512], f8, name="w2a")
              nc.gpsimd.dma_start(out=bfc1_sb, in_=bfc1)
              nc.gpsimd.dma_start(out=bfc2_bc, in_=bfc2)
              nc.gpsimd.dma_start(out=w2a, in_=wfc2_pk[:, 0])
              with tc.tile_pool(name="phF", bufs=1) as phF:
                h2T2 = [phF.tile([P, KD, T // 2], f8, name="h2Ta"),
                        phF.tile([P, KD, T // 2], f8, name="h2Tb")]
                NEARLY = 8

                with tc.tile_pool(name="w1h", bufs=4) as w1h_pool, \
                     tc.tile_pool(name="ps_e", bufs=3,
                                  space="PSUM") as ps_e, \
                     tc.tile_pool(name="ps_f1h", bufs=2,
                                  space="PSUM") as ps_f1h:

                    def fc1_half(w_pool, ps_pool, ff, nh):
                        w_t = w_pool.tile([P, KD, P], f8, name="w1_t")
                        nc.sync.dma_start(out=w_t, in_=wfc1_pk[:, ff])
                        ps = ps_pool.tile([P, 512], f32, name="ps_half",
                                          bufs=2)
                        for k in range(0, KD, 2):
                            nc.tensor.matmul(
                                ps, w_t[:, k:k + 2, :],
                                h2T2[nh][:, k:k + 2, :],
                                start=(k == 0), stop=(k == KD - 2),
                                perf_mode=DR)
                        nc.scalar.activation(
                            out=aT[:, ff, nh * 512:(nh + 1) * 512],
                            in_=ps, func=AF.Gelu, scale=2.0 ** -7,
                            bias=bfc1_sb[:, ff:ff + 1])

                    # proj sweep first: with only the cheap evictions on
                    # DVE the proj stream runs PE-bound back-to-back; the
                    # LN2 stats chains (the expensive DVE work) follow in a
                    # second pipelined sweep so they never gate the proj
                    # PSUM ring
                    x_rs = {}
                    for tt in range(3):
                        x_rs[tt] = xr_pool.tile([P, D], f32, name="x_r")
                        nc.sync.dma_start(out=x_rs[tt],
                                          in_=xb2[tt * P:(tt + 1) * P, :])
                    for tt in range(NT):
                        if tt in x_rs:
                            x_r = x_rs.pop(tt)
                        else:
                            x_r = xr_pool.tile([P, D], f32, name="x_r")
                            nc.sync.dma_start(out=x_r,
                                              in_=xb2[tt * P:(tt + 1) * P, :])
                        ps = ps_e.tile([P, T], f32, name="ps_mm")
                        for dh in range(2):
                            for k in range(0, KD, 2):
                                nc.tensor.matmul(
                                    ps[:, dh * 512:(dh + 1) * 512],
                                    oT[:, k:k + 2, tt * P:(tt + 1) * P],
                                    wp_sb[:, k:k + 2,
                                          dh * 512:(dh + 1) * 512],
                                    start=(k == 0), stop=(k == KD - 2),
                                    perf_mode=DR)
                        # x_r already carries x + proj bias (host-folded),
                        # so proj evict + bias + residual is one DVE op
                        nc.vector.scalar_tensor_tensor(
                            out=x2[:, tt, :], in0=ps, scalar=2.0 ** -7,
                            in1=x_r, op0=ALU.mult, op1=ALU.add)
                    h2_prev = None
                    for tt in range(NT):
                        h2_cur = ln_stats_norm(xr_pool, stat2_pool,
                                               x2[:, tt, :])
                        if h2_prev is not None:
                            ln_transpose(ps_e, h2_prev, h2T2[(tt - 1) // 4],
                                         (tt - 1) % 4)
                        h2_prev = h2_cur
                        if tt >= 4:
                            fc1_half(w1h_pool, ps_f1h, 2 * (tt - 4), 0)
                            fc1_half(w1h_pool, ps_f1h, 2 * (tt - 4) + 1, 0)
                    ln_transpose(ps_e, h2_prev, h2T2[1], 3)

                # ------- Phase G: the rest of fc1 -------
                with tc.tile_pool(name="w1", bufs=6) as w1_pool, \
                     tc.tile_pool(name="ps_f1", bufs=3,
                                  space="PSUM") as ps_f1:
                    for ff in range(NFF):
                        if ff < NEARLY:
                            fc1_half(w1_pool, ps_f1, ff, 1)
                            continue
                        w_t = w1_pool.tile([P, KD, P], f8, name="w1_t")
                        nc.sync.dma_start(out=w_t, in_=wfc1_pk[:, ff])
                        ps = ps_f1.tile([P, T], f32, name="ps_mm")
                        for nh in range(2):
                            for k in range(0, KD, 2):
                                nc.tensor.matmul(
                                    ps[:, nh * 512:(nh + 1) * 512],
                                    w_t[:, k:k + 2, :],
                                    h2T2[nh][:, k:k + 2, :],
                                    start=(k == 0), stop=(k == KD - 2),
                                    perf_mode=DR)
                        nc.scalar.activation(
                            out=aT[:, ff, :],
                            in_=ps, func=AF.Gelu, scale=2.0 ** -7,
                            bias=bfc1_sb[:, ff:ff + 1])

              # ------- Phase H: fc2 + residual -> out (h2T2 freed) -------
              with tc.tile_pool(name="w2b", bufs=1) as w2b_pool, \
                   tc.tile_pool(name="yb", bufs=3) as y_pool, \
                   tc.tile_pool(name="ps_f2", bufs=4,
                                space="PSUM") as ps_f2:
                w2b = w2b_pool.tile([P, NFF, 512], f8, name="w2b")
                nc.gpsimd.dma_start(out=w2b, in_=wfc2_pk[:, 1])
                for dh, w2_t in ((0, w2a), (1, w2b)):
                    sl = slice(dh * 512, (dh + 1) * 512)
                    for tt in range(NT):
                        ps = ps_f2.tile([P, 512], f32, name="ps_mm")
                        for k in range(0, NFF, 2):
                            nc.tensor.matmul(
                                ps, aT[:, k:k + 2, tt * P:(tt + 1) * P],
                                w2_t[:, k:k + 2, :],
                                start=(k == 0), stop=(k == NFF - 2),
                                perf_mode=DR)
                        y_sb = y_pool.tile([P, 512], f32, name="y_sb")
                        nc.vector.scalar_tensor_tensor(
                            out=y_sb, in0=ps, scalar=2.0 ** -8,
                            in1=bfc2_bc[:, sl], op0=ALU.mult, op1=ALU.add)
                        nc.vector.tensor_tensor(
                            out=y_sb, in0=y_sb,
                            in1=x2[:, tt, sl], op=ALU.add)
                        dma_eng = nc.sync if (tt + dh) % 2 == 0 \
                            else nc.gpsimd
                        dma_eng.dma_start(
                            out=out[tt * P:(tt + 1) * P, sl],
                            in_=y_sb)

    nc.compile()
    return nc


def _prep_host_inputs(x, ln1_g, ln1_b, ln2_g, ln2_b, qkv_w, q_bias, v_bias,
                      proj_w, proj_b, fc1_w, fc1_b, fc2_w, fc2_b):
    f32 = np.float32
    x = np.asarray(x, f32)
    ln1_g = np.asarray(ln1_g, f32)
    ln1_b = np.asarray(ln1_b, f32)
    ln2_g = np.asarray(ln2_g, f32)
    ln2_b = np.asarray(ln2_b, f32)
    qkv_w = np.asarray(qkv_w, f32)
    q_bias = np.asarray(q_bias, f32)
    v_bias = np.asarray(v_bias, f32)
    proj_w = np.asarray(proj_w, f32)
    proj_b = np.asarray(proj_b, f32)
    fc1_w = np.asarray(fc1_w, f32)
    fc1_b = np.asarray(fc1_b, f32)
    fc2_w = np.asarray(fc2_w, f32)
    fc2_b = np.asarray(fc2_b, f32)

    scale = HD ** (-0.5)
    # v_bias is a constant per-feature shift of o (softmax rows sum to 1),
    # so it folds into the proj bias: proj_b += proj_w @ v_bias.  The v part
    # of the qkv bias used on-chip is qkv_w_v @ ln1_b only.
    qkv_bias = np.concatenate(
        [q_bias, np.zeros_like(v_bias), np.zeros_like(v_bias)])
    wqkv = qkv_w * ln1_g[None, :]
    bqkv = qkv_w @ ln1_b + qkv_bias
    wqkv = wqkv.copy()
    wqkv[:D] *= scale
    bqkv[:D] *= scale
    proj_b = proj_b + proj_w @ (qkv_w[2 * D:] @ ln1_b + v_bias)

    wfc1 = fc1_w * ln2_g[None, :]
    bfc1 = fc1_w @ ln2_b + fc1_b

    # qkv/fc1/fc2 weights are cast to fp8e4m3 with power-of-two pre-scales
    # (q rows carry an extra 1/8 from the attention scale fold); the kernel
    # multiplies the matching 2^-s back in during psum eviction.  All weight
    # matrices are repacked partition-major so each SBUF tile loads with one
    # DMA of large contiguous per-partition lines.
    f8 = ml_dtypes.float8_e4m3
    KD_, NFF_, NF_ = D // 128, FF // 128, 3 * D // 128
    qkv_colscale = np.concatenate(
        [np.full(D, 2.0 ** 10, f32), np.full(2 * D, 2.0 ** 7, f32)])
    wqkvT8 = (wqkv.T * qkv_colscale[None, :]).astype(f8)
    wqkv_pk = wqkvT8[:, :2 * D].reshape(KD_, 128, 2 * KD_, 128) \
        .transpose(1, 2, 0, 3)
    wv_pk = wqkvT8[:, 2 * D:].reshape(KD_, 128, D).transpose(1, 0, 2)
    wp_pk = (proj_w.T * 2.0 ** 7).astype(f8) \
        .reshape(KD_, 128, D).transpose(1, 0, 2)
    wfc1_pk = (wfc1.T * 2.0 ** 7).astype(f8) \
        .reshape(KD_, 128, NFF_, 128).transpose(1, 2, 0, 3)
    wfc2_pk = (fc2_w.T * 2.0 ** 8).astype(f8) \
        .reshape(NFF_, 128, 2, 512).transpose(1, 2, 0, 3)
    shared = {
        "wqkv_pk": np.ascontiguousarray(wqkv_pk),
        "wv_pk": np.ascontiguousarray(wv_pk),
        "bqkv": np.ascontiguousarray(bqkv.reshape(NF_, 128).T, f32),
        "wp_pk": np.ascontiguousarray(wp_pk),
        "wfc1_pk": np.ascontiguousarray(wfc1_pk),
        "bfc1": np.ascontiguousarray(bfc1.reshape(NFF_, 128).T, f32),
        "wfc2_pk": np.ascontiguousarray(wfc2_pk),
        "bfc2": np.ascontiguousarray(
            np.broadcast_to(fc2_b, (128, D)), f32),
    }
    # residual stream with the proj bias pre-added (saves an on-chip add)
    x2r = x + proj_b[None, None, :]
    in_maps = [dict(shared, xb=np.ascontiguousarray(x[i]),
                    xb2=np.ascontiguousarray(x2r[i], f32))
               for i in range(N_CORES)]
    return in_maps


def kernel(**inputs):
    from concourse.bass_utils import run_bass_kernel_spmd

    if "nc" not in _CACHE:
        _CACHE["nc"] = _build_nc()
    nc = _CACHE["nc"]
    in_maps = _prep_host_inputs(**inputs)
    res = run_bass_kernel_spmd(nc, in_maps, core_ids=list(range(N_CORES)),
                               trace=False)
    return np.stack([res.results[i]["out"] for i in range(N_CORES)], axis=0)


if __name__ == "__main__":
    rng = np.random.default_rng(0)
    ins = {
        "x": rng.standard_normal((B, T, D)).astype(np.float32),
        "ln1_g": np.ones(D, np.float32), "ln1_b": np.zeros(D, np.float32),
        "ln2_g": np.ones(D, np.float32), "ln2_b": np.zeros(D, np.float32),
        "qkv_w": (rng.uniform(-1, 1, (3 * D, D)) / 32).astype(np.float32),
        "q_bias": np.zeros(D, np.float32), "v_bias": np.zeros(D, np.float32),
        "proj_w": (rng.uniform(-1, 1, (D, D)) / 32).astype(np.float32),
        "proj_b": np.zeros(D, np.float32),
        "fc1_w": (rng.uniform(-1, 1, (FF, D)) / 32).astype(np.float32),
        "fc1_b": np.zeros(FF, np.float32),
        "fc2_w": (rng.uniform(-1, 1, (D, FF)) / 64).astype(np.float32),
        "fc2_b": np.zeros(D, np.float32),
    }
    y = kernel(**ins)
    print("out", y.shape, y.dtype, np.abs(y).max())
